# revision 23
# baseline (speedup 1.0000x reference)
"""BasedAttention Trainium2 kernel — nn_BasedAttention_82214263980185.

Sharding: 8 NeuronCores = 2 batches x 4 head-groups (4 heads each). No
collectives: each core computes its head-group's contribution to the output
projection (a (2048, 1024) partial, bf16) and the host sums 4 partials per
batch and adds the residual.

Device math per core (T=2048, D=1024, 4 heads, head_dim=64, feat=16):
  - RMSNorm in T-major fp32, cast to bf16, PE-transpose to hT (D-major).
  - q/k produced feature-major (256, T) via lhsT=W; v produced T-major.
  - Linear attention: intra-chunk scores use the exact identity
      phi(a) . phi(b) = (1 + a.b/2)^2
    (one Square activation on qf.kf Gram chunk); inter-chunk state kv
    (153 x 65, incl. k-sum column) is kept in feature space. The Taylor
    quadratic scale (0.25 diag / 0.5 off-diag) is folded into the q-side
    features only (inner products are invariant to the u*w split).
  - Sliding-window attention computed transposed (scores (k, q)); no
    max-subtraction (|score/8| < ~15 so fp32 exp is safe); band mask is
    multiplicative after exp; the PV matmul's rhs carries a ones column so
    the softmax denominator falls out of the same matmul.
  - cat = [lin | win] assembled T-major, PE-transposed, multiplied by the
    head-group's Wout rows.
"""

import math
import os
import sys

sys.path.insert(0, "/opt/trn_rl_repo")

import numpy as np
import ml_dtypes

BF16 = ml_dtypes.bfloat16

P = 128
T = 2048
D = 1024
NH = 4          # heads per core
HD = 64
FEAT = 16
F = 153         # 1 + 16 + 136
NQUAD = 136
NT = 16         # T tiles / chunks
ND = 8          # D blocks
NSPAN = 4       # spans of 512
WINDOW = 64
EPS_NORM = 1e-6
EPS_LIN = 1e-6

_MODULE_CACHE = {}


def _build_module():
    import concourse.bass as bass
    from concourse import bacc
    import concourse.tile as tile
    from concourse import mybir
    from concourse.tile import add_dep_helper

    dtf = mybir.dt.float32
    dtb = mybir.dt.bfloat16
    AF = mybir.ActivationFunctionType
    OP = mybir.AluOpType

    import os as _os
    PH = _os.environ.get("KERNEL_PHASES", "full")
    do_feat = PH in ("feat", "lin", "win", "full")
    do_lin = PH in ("lin", "full")
    do_win = PH in ("win", "full")

    # Bacc (not raw Bass): its compile() pass splits multi-wait instructions
    # into event-semaphore chains — TRN2 allows at most one wait per inst.
    nc = bacc.Bacc()

    x_p = nc.declare_dram_parameter("x", [T, D], dtf, isOutput=False)
    wqkv_p = nc.declare_dram_parameter("wqkv", [P, ND, 768], dtb, isOutput=False)
    wf_p = nc.declare_dram_parameter("wf", [P, 32], dtb, isOutput=False)
    wout_p = nc.declare_dram_parameter("wout", [P, 4, D], dtb, isOutput=False)
    ident_p = nc.declare_dram_parameter("ident", [P, P], dtb, isOutput=False)
    tril_p = nc.declare_dram_parameter("tril", [P, P], dtb, isOutput=False)
    maskb_p = nc.declare_dram_parameter("maskb", [P, P], dtb, isOutput=False)
    maska_p = nc.declare_dram_parameter("maska", [P, P], dtb, isOutput=False)
    s2a_p = nc.declare_dram_parameter("s2a", [P, 1], dtf, isOutput=False)
    s2b_p = nc.declare_dram_parameter("s2b", [P, 1], dtf, isOutput=False)
    out_p = nc.declare_dram_parameter("out", [T, D], dtb, isOutput=True)

    with tile.TileContext(nc) as tc:
        from contextlib import ExitStack

        with ExitStack() as ctx:
            consts = ctx.enter_context(tc.tile_pool(name="consts", bufs=1))
            pers = ctx.enter_context(tc.tile_pool(name="pers", bufs=1))
            xp = ctx.enter_context(tc.tile_pool(name="xp", bufs=3))
            sqp = ctx.enter_context(tc.tile_pool(name="sqp", bufs=2))
            hbp = ctx.enter_context(tc.tile_pool(name="hbp", bufs=2))
            hTp = ctx.enter_context(tc.tile_pool(name="hTp", bufs=2))
            stp = ctx.enter_context(tc.tile_pool(name="stp", bufs=4))
            kvsb = ctx.enter_context(tc.tile_pool(name="kvsb", bufs=2))
            qpp = ctx.enter_context(tc.tile_pool(name="qpp", bufs=2))
            atp = ctx.enter_context(tc.tile_pool(name="atp", bufs=2))
            pwp = ctx.enter_context(tc.tile_pool(name="pwp", bufs=2))
            ctp = ctx.enter_context(tc.tile_pool(name="ctp", bufs=2))
            obp = ctx.enter_context(tc.tile_pool(name="obp", bufs=2))

            ps_tp = ctx.enter_context(tc.tile_pool(name="ps_tp", bufs=2, space="PSUM"))
            ps_mm = ctx.enter_context(tc.tile_pool(name="ps_mm", bufs=2, space="PSUM"))
            ps_at = ctx.enter_context(tc.tile_pool(name="ps_at", bufs=2, space="PSUM"))
            ps_kv = ctx.enter_context(tc.tile_pool(name="ps_kv", bufs=2, space="PSUM"))

            # ---- constants ----
            wqkv_sb = consts.tile([P, ND, 768], dtb)
            nc.gpsimd.dma_start(out=wqkv_sb[:], in_=wqkv_p[:])
            wf_sb = consts.tile([P, 32], dtb)
            nc.gpsimd.dma_start(out=wf_sb[:], in_=wf_p[:])
            wout_sb = consts.tile([P, 4, D], dtb)
            nc.gpsimd.dma_start(out=wout_sb[:], in_=wout_p[:])
            ident = consts.tile([P, P], dtb)
            nc.gpsimd.dma_start(out=ident[:], in_=ident_p[:])
            tril = consts.tile([P, P], dtb)
            nc.gpsimd.dma_start(out=tril[:], in_=tril_p[:])
            maskb = consts.tile([P, P], dtb)
            nc.gpsimd.dma_start(out=maskb[:], in_=maskb_p[:])
            maska = consts.tile([P, P], dtb)
            nc.gpsimd.dma_start(out=maska[:], in_=maska_p[:])
            s2a = consts.tile([P, 1], dtf)
            nc.gpsimd.dma_start(out=s2a[:], in_=s2a_p[:])
            s2b = consts.tile([P, 1], dtf)
            nc.gpsimd.dma_start(out=s2b[:], in_=s2b_p[:])
            epsn = consts.tile([P, 1], dtf)
            nc.vector.memset(epsn[:], EPS_NORM)

            # ---- persistent tensors ----
            qT = [pers.tile([P, 2, 512], dtb, tag=f"qT{s}", name=f"qT{s}") for s in range(NSPAN)]
            kT = [pers.tile([P, 2, 512], dtb, tag=f"kT{s}", name=f"kT{s}") for s in range(NSPAN)]
            vones = [pers.tile([P, NH, 65], dtb, tag=f"vo{c}", name=f"vo{c}") for c in range(NT)]
            qkfT = [[pers.tile([64, 2, 512], dtb, tag=f"qkfT{s}_{hp}", name=f"qkfT{s}_{hp}")
                     for hp in range(2)] for s in range(NSPAN)]
            qkf_sm = pers.tile([P, NT, 2, NH, FEAT], dtb, tag="qkf_sm")
            phi_sm = pers.tile([P, NT, 2, NH, F], dtb, tag="phi_sm")
            cat = [pers.tile([P, 512], dtb, tag=f"cat{c}", name=f"cat{c}") for c in range(NT)]

            for c in range(NT):
                nc.vector.memset(vones[c][:, :, 64:65], 1.0)
                if PH != "full":
                    nc.vector.memset(cat[c][:], 0.0)
            for s in range(NSPAN):
                for hp in range(2):
                    nc.vector.memset(qkfT[s][hp][:], 0.0)

            # ---- phase 1+2: rmsnorm, transpose h, QKV projections ----
            for s in range(NSPAN):
                hT = hTp.tile([P, ND, 512], dtb)
                for tt in range(4):
                    t = s * 4 + tt
                    x_t = xp.tile([P, D], dtf)
                    nc.gpsimd.dma_start(out=x_t[:], in_=x_p[t * P:(t + 1) * P, :])
                    sqt = sqp.tile([P, D], dtf)
                    ss = stp.tile([P, 1], dtf, tag="ss")
                    nc.vector.tensor_mul(sqt[:], x_t[:], x_t[:])
                    nc.vector.reduce_sum(ss[:], sqt[:], axis=mybir.AxisListType.X)
                    rms = stp.tile([P, 1], dtf, tag="rms")
                    nc.scalar.activation(rms[:], ss[:], AF.Sqrt,
                                         bias=epsn[:], scale=1.0 / D)
                    rinv = stp.tile([P, 1], dtf, tag="rinv")
                    nc.vector.reciprocal(rinv[:], rms[:])
                    h_bf = hbp.tile([P, D], dtb)
                    nc.vector.tensor_scalar_mul(h_bf[:], x_t[:], rinv[:])
                    for db in range(ND):
                        tp = ps_tp.tile([P, P], dtb, tag="tp")
                        nc.tensor.transpose(tp[:], h_bf[:, db * P:(db + 1) * P], ident[:])
                        nc.vector.tensor_copy(hT[:, db, tt * P:(tt + 1) * P], tp[:])
                # q and k: feature-major (lhsT = weight block)
                for proj, dst in ((0, qT), (1, kT)):
                    for mb in range(2):
                        pq = ps_mm.tile([P, 512], dtf, tag="mm")
                        for db in range(ND):
                            nc.tensor.matmul(
                                pq[:],
                                lhsT=wqkv_sb[:, db, proj * 256 + mb * P:proj * 256 + (mb + 1) * P],
                                rhs=hT[:, db, :],
                                start=(db == 0), stop=(db == ND - 1))
                        nc.vector.tensor_copy(dst[s][:, mb, :], pq[:])
                # v: T-major
                for tt in range(4):
                    t = s * 4 + tt
                    pv = ps_mm.tile([P, 256], dtf, tag="mm")
                    for db in range(ND):
                        nc.tensor.matmul(
                            pv[:],
                            lhsT=hT[:, db, tt * P:(tt + 1) * P],
                            rhs=wqkv_sb[:, db, 512:768],
                            start=(db == 0), stop=(db == ND - 1))
                    nc.vector.tensor_copy(
                        vones[t][:, :, 0:64],
                        pv[:].rearrange("p (h d) -> p h d", h=NH))

            # ---- phase 3: qf/kf features ----
            for side in range(2 if do_feat else 0):
                for s in range(NSPAN):
                    srcqk = qT if side == 0 else kT
                    for hp in range(2):
                        pf = ps_mm.tile([64, 512], dtf, tag="mm", name="pf")
                        for ho in range(2):
                            h = hp * 2 + ho
                            hr = (h % 2) * 64
                            nc.tensor.matmul(
                                pf[ho * 32:ho * 32 + FEAT, :],
                                lhsT=wf_sb[hr:hr + 64, side * FEAT:(side + 1) * FEAT],
                                rhs=srcqk[s][hr:hr + 64, h // 2, :],
                                start=True, stop=True)
                            nc.vector.tensor_copy(
                                qkfT[s][hp][ho * 32:ho * 32 + FEAT, side, :],
                                pf[ho * 32:ho * 32 + FEAT, :])
            # s-major qf/kf via transpose (4 heads packed per transpose)
            for side in range(2 if do_feat else 0):
                for tb in range(NT):
                    s, col = tb // 4, (tb % 4) * P
                    for hp in range(2):
                        tpf = ps_tp.tile([P, 64], dtb, tag="tp", name="tpf")
                        nc.tensor.transpose(
                            tpf[:], qkfT[s][hp][:, side, col:col + P],
                            ident[0:64, 0:64])
                        nc.vector.tensor_copy(
                            qkf_sm[:, tb, side, hp * 2:hp * 2 + 2, :],
                            tpf[:].rearrange("p (h f) -> p h f", h=2)[:, :, 0:FEAT])

            # ---- taylor features, T-major, batched over tiles/sides/heads ----
            if do_feat:
              nc.vector.memset(phi_sm[:, :, :, :, 0:1], 1.0)
              nc.vector.tensor_copy(phi_sm[:, :, :, :, 1:1 + FEAT], qkf_sm[:])
              base = 1 + FEAT
              for i in range(FEAT):
                w = FEAT - i
                nc.vector.tensor_mul(
                    phi_sm[:, :, :, :, base:base + w],
                    qkf_sm[:, :, :, :, i:FEAT],
                    qkf_sm[:, :, :, :, i:i + 1].to_broadcast((P, NT, 2, NH, w)))
                base += w

            # ---- phase 4: attention ----
            for h in range(NH):
                hr = (h % 2) * 64
                hm = h // 2
                kv0_prev = None
                kv1_prev = None
                for c in range(NT):
                    s, col = c // 4, (c % 4) * P
                    if not do_lin:
                        break
                    # --- linear attention ---
                    # q_phi feature-major for this (h, c), q-side scale s2
                    qp0 = qpp.tile([P, P], dtb, tag="qp0")
                    qp1 = qpp.tile([25, P], dtb, tag="qp1")
                    tq0 = ps_tp.tile([P, P], dtb, tag="tp")
                    nc.tensor.transpose(tq0[:], phi_sm[:, c, 0, h, 0:P], ident[:])
                    nc.vector.tensor_scalar_mul(qp0[:], tq0[:], s2a[:])
                    tq1 = ps_tp.tile([25, P], dtb, tag="tp")
                    nc.tensor.transpose(tq1[:], phi_sm[:, c, 0, h, P:F], ident[:])
                    nc.vector.tensor_scalar_mul(qp1[:], tq1[:], s2b[0:25, :])
                    # intra-chunk scores: A_T = tril * (1 + G/2)^2
                    g = ps_at.tile([P, P], dtf, tag="at")
                    nc.tensor.matmul(
                        g[:],
                        lhsT=qkfT[s][h // 2][(h % 2) * 32:(h % 2) * 32 + FEAT, 1, col:col + P],
                        rhs=qkfT[s][h // 2][(h % 2) * 32:(h % 2) * 32 + FEAT, 0, col:col + P],
                        start=True, stop=True)
                    a_t = atp.tile([P, P], dtb)
                    nc.scalar.activation(a_t[:], g[:], AF.Square, bias=1.0, scale=0.5)
                    nc.vector.tensor_mul(a_t[:], a_t[:], tril[:])
                    # y/z accumulation
                    yz = ps_at.tile([P, 65], dtf, tag="at")
                    nc.tensor.matmul(yz[:], lhsT=a_t[:], rhs=vones[c][:, h, :],
                                     start=True, stop=(c == 0))
                    if c > 0:
                        nc.tensor.matmul(yz[:], lhsT=qp0[:], rhs=kv0_prev[:],
                                         start=False, stop=False)
                        nc.tensor.matmul(yz[:], lhsT=qp1[:], rhs=kv1_prev[:],
                                         start=False, stop=True)
                    # kv state update (accumulating across chunks in PSUM)
                    # kv state: per-chunk delta in PSUM (kv0/kv1 share one
                    # bank zero-region; start on first, stop on second), then
                    # chained bf16 accumulation in SBUF. The last chunk's
                    # update is never consumed, so skip it.
                    if c < NT - 1:
                        delta = ps_kv.tile([P, 130], dtf, tag="kv", name="delta",
                                           padded_shape=[P, 512])
                        nc.tensor.matmul(delta[:, 0:65],
                                         lhsT=phi_sm[:, c, 1, h, 0:P],
                                         rhs=vones[c][:, h, :],
                                         start=True, stop=True)
                        i1 = nc.tensor.matmul(delta[0:25, 65:130],
                                              lhsT=phi_sm[:, c, 1, h, P:F],
                                              rhs=vones[c][:, h, :],
                                              start=True, stop=True,
                                              skip_group_check=True)
                        kv0 = kvsb.tile([P, 65], dtb, tag="kv0")
                        kv1 = kvsb.tile([25, 65], dtb, tag="kv1")
                        if c == 0:
                            c0 = nc.vector.tensor_copy(kv0[:], delta[:, 0:65])
                            # kv0's region belongs to a zero-region whose group
                            # is closed by i1 — don't read before then.
                            add_dep_helper(c0.ins, i1.ins)
                            nc.vector.tensor_copy(kv1[:], delta[0:25, 65:130])
                        else:
                            c0 = nc.vector.tensor_add(kv0[:], kv0_prev[:],
                                                      delta[:, 0:65])
                            add_dep_helper(c0.ins, i1.ins)
                            nc.vector.tensor_add(kv1[:], kv1_prev[:],
                                                 delta[0:25, 65:130])
                        kv0_prev, kv1_prev = kv0, kv1
                    # normalize -> cat lin slice
                    rz = stp.tile([P, 1], dtf, tag="rz")
                    nc.vector.tensor_scalar_add(rz[:], yz[:, 64:65], EPS_LIN)
                    nc.vector.reciprocal(rz[:], rz[:])
                    nc.vector.tensor_scalar_mul(cat[c][:, h * 64:(h + 1) * 64],
                                                yz[:, 0:64], rz[:])

                    # --- sliding window attention (transposed scores) ---
                    if not do_win:
                        continue
                    yw = ps_at.tile([P, 65], dtf, tag="at")
                    if c == 0:
                        s0 = ps_at.tile([P, P], dtf, tag="at")
                        nc.tensor.matmul(
                            s0[:],
                            lhsT=kT[0][hr:hr + 64, hm, 0:P],
                            rhs=qT[0][hr:hr + 64, hm, 0:P],
                            start=True, stop=True)
                        p0 = pwp.tile([P, P], dtb, tag="pB")
                        nc.scalar.activation(p0[:], s0[:], AF.Exp, scale=0.125)
                        nc.vector.tensor_mul(p0[:], p0[:], maskb[:])
                        nc.tensor.matmul(yw[:], lhsT=p0[:], rhs=vones[0][:, h, :],
                                         start=True, stop=True)
                    else:
                        sa = ps_at.tile([P, P], dtf, tag="at")
                        sprev, colprev = (c - 1) // 4, ((c - 1) % 4) * P
                        nc.tensor.matmul(
                            sa[64:P, :],
                            lhsT=kT[sprev][hr:hr + 64, hm, colprev + 64:colprev + P],
                            rhs=qT[s][hr:hr + 64, hm, col:col + P],
                            start=True, stop=True)
                        sb_ = ps_at.tile([P, P], dtf, tag="at")
                        nc.tensor.matmul(
                            sb_[:],
                            lhsT=kT[s][hr:hr + 64, hm, col:col + P],
                            rhs=qT[s][hr:hr + 64, hm, col:col + P],
                            start=True, stop=True)
                        pa = pwp.tile([P, P], dtb, tag="pA")
                        nc.scalar.activation(pa[64:P, :], sa[64:P, :], AF.Exp, scale=0.125)
                        nc.vector.tensor_mul(pa[64:P, :], pa[64:P, :], maska[64:P, :])
                        pb = pwp.tile([P, P], dtb, tag="pB")
                        nc.scalar.activation(pb[:], sb_[:], AF.Exp, scale=0.125)
                        nc.vector.tensor_mul(pb[:], pb[:], maskb[:])
                        nc.tensor.matmul(yw[:], lhsT=pa[64:P, :],
                                         rhs=vones[c - 1][64:P, h, :],
                                         start=True, stop=False)
                        nc.tensor.matmul(yw[:], lhsT=pb[:], rhs=vones[c][:, h, :],
                                         start=False, stop=True)
                    rw = stp.tile([P, 1], dtf, tag="rw")
                    nc.vector.reciprocal(rw[:], yw[:, 64:65])
                    nc.vector.tensor_scalar_mul(cat[c][:, 256 + h * 64:256 + (h + 1) * 64],
                                                yw[:, 0:64], rw[:])

            # ---- phase 5: output projection ----
            for c in range(NT):
                catT = ctp.tile([P, 4, P], dtb)
                for fb in range(4):
                    tpc = ps_tp.tile([P, P], dtb, tag="tp")
                    nc.tensor.transpose(tpc[:], cat[c][:, fb * P:(fb + 1) * P], ident[:])
                    nc.vector.tensor_copy(catT[:, fb, :], tpc[:])
                ob = obp.tile([P, D], dtb)
                for nb in range(2):
                    po = ps_mm.tile([P, 512], dtf, tag="mm")
                    for fb in range(4):
                        nc.tensor.matmul(po[:], lhsT=catT[:, fb, :],
                                         rhs=wout_sb[:, fb, nb * 512:(nb + 1) * 512],
                                         start=(fb == 0), stop=(fb == 3))
                    nc.vector.tensor_copy(ob[:, nb * 512:(nb + 1) * 512], po[:])
                nc.gpsimd.dma_start(out=out_p[c * P:(c + 1) * P, :], in_=ob[:])

    nc.compile()
    return nc


def _host_inputs(x, norm_w, Wq, Wk, Wv, Wqf, Wkf, Wout):
    """Build the 8 per-core input maps."""
    x = np.asarray(x, np.float32)
    norm_w = np.asarray(norm_w, np.float32)
    Wqn = (np.asarray(Wq, np.float32) * norm_w[:, None])
    Wkn = (np.asarray(Wk, np.float32) * norm_w[:, None])
    Wvn = (np.asarray(Wv, np.float32) * norm_w[:, None])
    Wqf = np.asarray(Wqf, np.float32)
    Wkf = np.asarray(Wkf, np.float32)
    Wout = np.asarray(Wout, np.float32)

    wf_half = np.concatenate([Wqf, Wkf], axis=1)
    wf = np.concatenate([wf_half, wf_half], axis=0).astype(BF16)
    ident = np.eye(P, dtype=np.float32).astype(BF16)
    sidx = np.arange(P)[:, None]
    ridx = np.arange(P)[None, :]
    tril = (sidx <= ridx).astype(np.float32).astype(BF16)
    maskb = ((sidx <= ridx) & (ridx <= sidx + WINDOW)).astype(np.float32).astype(BF16)
    maska = np.zeros((P, P), np.float32)
    sa_idx = np.arange(WINDOW)[:, None]
    maska[64:P, :] = (ridx <= sa_idx).astype(np.float32)
    maska = maska.astype(BF16)
    iu, ju = np.triu_indices(FEAT)
    s2vec = np.where(iu == ju, 0.25, 0.5).astype(np.float32)
    s2a = np.concatenate([np.ones(1 + FEAT, np.float32), s2vec[:P - (1 + FEAT)]])
    s2b = np.zeros(P, np.float32)
    s2b[:F - P] = s2vec[P - (1 + FEAT):]

    in_maps = []
    for core in range(8):
        b, g = core // 4, core % 4
        cols = slice(g * 256, (g + 1) * 256)
        wqkv = np.concatenate([Wqn[:, cols], Wkn[:, cols], Wvn[:, cols]], axis=1)
        wqkv = wqkv.reshape(ND, P, 768).transpose(1, 0, 2).astype(BF16)
        wrows = np.concatenate([Wout[g * 256:(g + 1) * 256],
                                Wout[D + g * 256:D + (g + 1) * 256]], axis=0)
        wout = wrows.reshape(4, P, D).transpose(1, 0, 2).astype(BF16)
        in_maps.append({
            "x": np.ascontiguousarray(x[b]),
            "wqkv": np.ascontiguousarray(wqkv),
            "wf": wf,
            "wout": np.ascontiguousarray(wout),
            "ident": ident,
            "tril": tril,
            "maskb": maskb,
            "maska": maska,
            "s2a": s2a[:, None],
            "s2b": s2b[:, None],
        })
    return x, in_maps


def kernel(x, norm_w, Wq, Wk, Wv, Wqf, Wkf, Wout):
    import jax

    # Persistent compilation cache: the NEFF compile is minutes of wall time;
    # reuse it across processes when possible.
    try:
        if jax.config.jax_compilation_cache_dir is None:
            jax.config.update("jax_compilation_cache_dir",
                              "/tmp/jax_neff_cache")
            jax.config.update("jax_persistent_cache_min_compile_time_secs", 1.0)
            jax.config.update("jax_persistent_cache_min_entry_size_bytes", 0)
    except Exception:
        pass

    from concourse.bass_utils import run_bass_kernel_spmd

    x, in_maps = _host_inputs(x, norm_w, Wq, Wk, Wv, Wqf, Wkf, Wout)

    if "nc" not in _MODULE_CACHE:
        _MODULE_CACHE["nc"] = _build_module()
    nc = _MODULE_CACHE["nc"]

    res = run_bass_kernel_spmd(nc, in_maps, core_ids=list(range(8)))
    if res.exec_time_ns is not None:
        kernel.last_exec_time_ns = int(res.exec_time_ns)

    out = np.empty((2, T, D), np.float32)
    for b in range(2):
        acc = x[b].copy()
        for g in range(4):
            acc += res.results[b * 4 + g]["out"].astype(np.float32)
        out[b] = acc
    return out


kernel.last_exec_time_ns = None
